# revision 7
# baseline (speedup 1.0000x reference)
"""Trainium2 Bass kernel for a top-1 (switch) MoE layer.

Strategy (expert-parallel, 8 experts -> 8 cores):
  * Host: router matmul (f64, cast f32), softmax/argmax, sort tokens by
    expert, pad each expert's token set to a common capacity C.
  * Device (SPMD, one expert per core): y = relu(x @ w1_e) @ w2_e for the
    tokens routed to expert e.  Two chained matmuls on the tensor engine in
    float32r (full-rate fp32 mode), relu on the scalar engine, PSUM
    accumulation over the contraction dims.
  * Host: unpermute, scale by router prob, return (out, logits, expert_idx).
"""

import math
import os
from contextlib import ExitStack

import numpy as np

import concourse.bass as bass
import concourse.mybir as mybir
import concourse.tile as tile
from concourse import bacc
from concourse.bass_utils import run_bass_kernel_spmd

B, S, D, E, F = 2, 2048, 768, 8, 3072
T = B * S
KD, KF = D // 128, F // 128  # 6, 24
NCORES = 8

# matmul input mode: "f32r" (full-rate fp32), "f32" (1/4-rate), "bf16"
MM_MODE = os.environ.get("MOE_MM_MODE", "f32r")


def _chunks_for(C):
    """Split [0, C) into contiguous chunks each <=512 (PSUM bank limit),
    sized as evenly as possible (keeps f32r at full rate for C >= 512)."""
    n = max(1, math.ceil(C / 512))
    base = C // n
    rem = C - base * n
    out, start = [], 0
    for i in range(n):
        cn = base + (1 if i < rem else 0)
        out.append((start, cn))
        start += cn
    return out


def _build_program(C):
    """Build the per-core Bass program for capacity C tokens."""
    if MM_MODE == "bf16":
        io_dt = mybir.dt.bfloat16
    elif MM_MODE == "f32r":
        io_dt = mybir.dt.float32r
    else:
        io_dt = mybir.dt.float32
    mm_dt = io_dt

    nc = bacc.Bacc(
        "TRN2", target_bir_lowering=False, debug=False, num_devices=NCORES
    )
    xT = nc.dram_tensor("xT", [KD, 128, C], io_dt, kind="ExternalInput").ap()
    w1 = nc.dram_tensor("w1", [KD, 128, F], io_dt, kind="ExternalInput").ap()
    w2 = nc.dram_tensor("w2", [KF, 128, D], io_dt, kind="ExternalInput").ap()
    yT = nc.dram_tensor(
        "yT", [KD, 128, C], mybir.dt.float32, kind="ExternalOutput"
    ).ap()

    chunks = _chunks_for(C)
    G = 6  # w1 f-tiles per load group (one [128, 768] DMA)

    with tile.TileContext(nc) as tc, ExitStack() as ctx:
        xpool = ctx.enter_context(tc.tile_pool(name="x", bufs=1))
        w1pool = ctx.enter_context(tc.tile_pool(name="w1", bufs=1))
        w2pool = ctx.enter_context(tc.tile_pool(name="w2", bufs=1))
        h1pool = ctx.enter_context(tc.tile_pool(name="h1", bufs=4))
        evpool = ctx.enter_context(tc.tile_pool(name="ev", bufs=4))
        phpool = ctx.enter_context(tc.tile_pool(name="ph", bufs=2, space="PSUM"))
        pypool = ctx.enter_context(tc.tile_pool(name="py", bufs=1, space="PSUM"))

        # Resident loads: x (all tokens, transposed), w1, then w2 streamed in
        # consumption order.
        x_t = []
        for k in range(KD):
            t = xpool.tile([128, C], io_dt, tag=f"x{k}", name=f"x{k}")
            nc.sync.dma_start(t[:], xT[k])
            x_t.append(t)
        w1_t = {}
        for g in range(KF // G):
            for k in range(KD):
                t = w1pool.tile([128, G * 128], io_dt, tag=f"w1_{k}_{g}", name=f"w1_{k}_{g}")
                nc.sync.dma_start(t[:], w1[k, :, g * G * 128 : (g + 1) * G * 128])
                w1_t[(k, g)] = t
        w2_t = []
        for f in range(KF):
            t = w2pool.tile([128, D], io_dt, tag=f"w2_{f}", name=f"w2_{f}")
            nc.sync.dma_start(t[:], w2[f])
            w2_t.append(t)

        for c0, cn in chunks:
            py_t = [
                pypool.tile([128, cn], mybir.dt.float32, tag=f"py{d}", name=f"py{d}")
                for d in range(KD)
            ]
            h1_live = {}
            # Software-pipelined: emit mm1(f) one step ahead of mm2(f-1) so
            # the PE never stalls on the relu between the two layers.
            for f in range(KF + 1):
                if f < KF:
                    ph = phpool.tile([128, cn], mybir.dt.float32, tag="ph", name=f"ph{f}")
                    g, j = divmod(f, G)
                    for k in range(KD):
                        nc.tensor.matmul(
                            ph[:],
                            w1_t[(k, g)][:, j * 128 : (j + 1) * 128],
                            x_t[k][:, c0 : c0 + cn],
                            start=(k == 0),
                            stop=(k == KD - 1),
                        )
                    h1 = h1pool.tile([128, cn], io_dt, tag="h1", name=f"h1_{f}")
                    nc.scalar.activation(
                        h1[:], ph[:], mybir.ActivationFunctionType.Relu
                    )
                    h1_live[f] = h1
                if f >= 1:
                    fp = f - 1
                    h1p = h1_live.pop(fp)
                    for d in range(KD):
                        nc.tensor.matmul(
                            py_t[d][:],
                            w2_t[fp][:, d * 128 : (d + 1) * 128],
                            h1p[:],
                            start=(fp == 0),
                            stop=(fp == KF - 1),
                        )
            for d in range(KD):
                ev = evpool.tile([128, cn], mybir.dt.float32, tag="ev", name=f"ev{d}")
                nc.vector.tensor_copy(ev[:], py_t[d][:])
                nc.sync.dma_start(yT[d, :, c0 : c0 + cn], ev[:])

    nc.compile()
    return nc


def _route_host(hidden_states, w_router):
    """Replicate the reference router on host (f64 -> f32 rounding)."""
    x = np.ascontiguousarray(hidden_states, dtype=np.float32).reshape(T, D)
    l64 = x.astype(np.float64) @ np.asarray(w_router, np.float64)
    logits = l64.astype(np.float32)
    eidx = np.argmax(logits, axis=1).astype(np.int32)
    m = l64.max(axis=1, keepdims=True)
    p = np.exp(l64 - m)
    pmax = (p.max(axis=1) / p.sum(axis=1)).astype(np.float32)
    return x, logits, eidx, pmax


def prepare(hidden_states, w_router, w1, w2):
    """Host routing + input sharding + program build (no execution)."""
    x, logits, eidx, pmax = _route_host(hidden_states, w_router)

    order = np.argsort(eidx, kind="stable")
    counts = np.bincount(eidx, minlength=E)
    C = max(256, int(math.ceil(counts.max() / 32)) * 32)

    np_io = np.float32
    if MM_MODE == "bf16":
        import ml_dtypes

        np_io = ml_dtypes.bfloat16

    tok_of_expert = np.split(order, np.cumsum(counts)[:-1])
    in_maps = []
    w1r = np.ascontiguousarray(w1, np.float32).reshape(E, KD, 128, F)
    w2r = np.ascontiguousarray(w2, np.float32).reshape(E, KF, 128, D)
    for e in range(E):
        xe = np.zeros((D, C), np.float32)
        xe[:, : counts[e]] = x[tok_of_expert[e]].T
        in_maps.append(
            {
                "xT": xe.reshape(KD, 128, C).astype(np_io),
                "w1": w1r[e].astype(np_io),
                "w2": w2r[e].astype(np_io),
            }
        )

    nc = _build_program(C)
    meta = dict(
        C=C, counts=counts, tok_of_expert=tok_of_expert, logits=logits,
        eidx=eidx, pmax=pmax,
    )
    return nc, in_maps, meta


def finish(results, meta):
    """Gather per-core outputs back into the full [B,S,D] output."""
    C, counts, tok_of_expert = meta["C"], meta["counts"], meta["tok_of_expert"]
    y = np.empty((T, D), np.float32)
    for e in range(E):
        yT = np.asarray(results[e]["yT"], np.float32)
        y[tok_of_expert[e]] = yT.reshape(D, C).T[: counts[e]]
    out = (meta["pmax"][:, None] * y).reshape(B, S, D).astype(np.float32)
    return out


def run(hidden_states, w_router, w1, w2, trace=False, trace_cores=None):
    nc, in_maps, meta = prepare(hidden_states, w_router, w1, w2)
    res = run_bass_kernel_spmd(
        nc,
        in_maps,
        list(range(NCORES)),
        trace=trace,
        trace_cores=trace_cores,
    )

    out = finish(res.results, meta)
    return (
        (out, meta["logits"].reshape(B, S, E), meta["eidx"].reshape(B, S)),
        res,
    )


def kernel(hidden_states, w_router, w1, w2):
    outs, _ = run(hidden_states, w_router, w1, w2)
    return outs


# revision 11
# speedup vs baseline: 74.5067x; 74.5067x over previous
"""Trainium2 Bass kernel for a top-1 (switch) MoE layer.

Strategy (expert-parallel, 8 experts -> 8 cores):
  * Host: router matmul (f64, cast f32), softmax/argmax, sort tokens by
    expert, pad each expert's token set to a common capacity C.
  * Device (SPMD, one expert per core): y = relu(x @ w1_e) @ w2_e for the
    tokens routed to expert e.  Two chained matmuls on the tensor engine in
    float32r (full-rate fp32 mode), relu on the scalar engine, PSUM
    accumulation over the contraction dims.
  * Host: unpermute, scale by router prob, return (out, logits, expert_idx).
"""

import math
import os
from contextlib import ExitStack

import numpy as np

import concourse.bass as bass
import concourse.mybir as mybir
import concourse.tile as tile
from concourse import bacc
from concourse.bass_utils import run_bass_kernel_spmd

B, S, D, E, F = 2, 2048, 768, 8, 3072
T = B * S
KD, KF = D // 128, F // 128  # 6, 24
NCORES = 8

# matmul input mode: "f32r" (full-rate fp32), "f32" (1/4-rate), "bf16"
MM_MODE = os.environ.get("MOE_MM_MODE", "f32r")


def _chunks_for(C):
    """Split [0, C) into contiguous chunks each <=512 (PSUM bank limit),
    sized as evenly as possible (keeps f32r at full rate for C >= 512)."""
    n = max(1, math.ceil(C / 512))
    base = C // n
    rem = C - base * n
    out, start = [], 0
    for i in range(n):
        cn = base + (1 if i < rem else 0)
        out.append((start, cn))
        start += cn
    return out


def _build_program(C, reps=1):
    """Build the per-core Bass program for capacity C tokens.

    reps > 1 repeats the whole load+compute+store body (for benchmarking:
    per-rep time = slope of launch time vs reps)."""
    if MM_MODE == "bf16":
        io_dt = mybir.dt.bfloat16
    elif MM_MODE == "f32r":
        io_dt = mybir.dt.float32r
    else:
        io_dt = mybir.dt.float32
    mm_dt = io_dt

    nc = bacc.Bacc(
        "TRN2", target_bir_lowering=False, debug=False, num_devices=NCORES
    )
    xT = nc.dram_tensor("xT", [KD, 128, C], io_dt, kind="ExternalInput").ap()
    w1 = nc.dram_tensor("w1", [KD, 128, F], io_dt, kind="ExternalInput").ap()
    w2 = nc.dram_tensor("w2", [KF, 128, D], io_dt, kind="ExternalInput").ap()
    yT = nc.dram_tensor(
        "yT", [KD, 128, C], mybir.dt.float32, kind="ExternalOutput"
    ).ap()

    chunks = _chunks_for(C)
    G = 6  # w1 f-tiles per load group (one [128, 768] DMA)

    with tile.TileContext(nc) as tc, ExitStack() as ctx:
        xpool = ctx.enter_context(tc.tile_pool(name="x", bufs=1))
        w1pool = ctx.enter_context(tc.tile_pool(name="w1", bufs=1))
        w2pool = ctx.enter_context(tc.tile_pool(name="w2", bufs=1))
        h1pool = ctx.enter_context(tc.tile_pool(name="h1", bufs=4))
        evpool = ctx.enter_context(tc.tile_pool(name="ev", bufs=4))
        phpool = ctx.enter_context(tc.tile_pool(name="ph", bufs=2, space="PSUM"))
        pypool = ctx.enter_context(tc.tile_pool(name="py", bufs=1, space="PSUM"))

        # Per rep: load x/w1/w2 (consumption order), compute, store.  Tags
        # are shared across reps so rep r+1 reuses (and waits for) rep r's
        # buffers -- reps measure steady-state end-to-end kernel time.
        for rep in range(reps):
            x_t = []
            for k in range(KD):
                t = xpool.tile([128, C], io_dt, tag=f"x{k}", name=f"x{k}_{rep}")
                nc.sync.dma_start(t[:], xT[k])
                x_t.append(t)
            w1_t = {}
            for g in range(KF // G):
                for k in range(KD):
                    t = w1pool.tile(
                        [128, G * 128], io_dt,
                        tag=f"w1_{k}_{g}", name=f"w1_{k}_{g}_{rep}",
                    )
                    nc.sync.dma_start(
                        t[:], w1[k, :, g * G * 128 : (g + 1) * G * 128]
                    )
                    w1_t[(k, g)] = t
            w2_t = []
            for f in range(KF):
                t = w2pool.tile(
                    [128, D], io_dt, tag=f"w2_{f}", name=f"w2_{f}_{rep}"
                )
                nc.sync.dma_start(t[:], w2[f])
                w2_t.append(t)

            for c0, cn in chunks:
                py_t = [
                    pypool.tile(
                        [128, cn], mybir.dt.float32,
                        tag=f"py{d}", name=f"py{d}_{rep}",
                    )
                    for d in range(KD)
                ]
                h1_live = {}
                # Software-pipelined: emit mm1(f) one step ahead of mm2(f-1)
                # so the PE never stalls on the relu between the two layers.
                for f in range(KF + 1):
                    if f < KF:
                        ph = phpool.tile(
                            [128, cn], mybir.dt.float32,
                            tag="ph", name=f"ph{f}_{rep}",
                        )
                        g, j = divmod(f, G)
                        for k in range(KD):
                            nc.tensor.matmul(
                                ph[:],
                                w1_t[(k, g)][:, j * 128 : (j + 1) * 128],
                                x_t[k][:, c0 : c0 + cn],
                                start=(k == 0),
                                stop=(k == KD - 1),
                            )
                        h1 = h1pool.tile(
                            [128, cn], io_dt, tag="h1", name=f"h1_{f}_{rep}"
                        )
                        nc.scalar.activation(
                            h1[:], ph[:], mybir.ActivationFunctionType.Relu
                        )
                        h1_live[f] = h1
                    if f >= 1:
                        fp = f - 1
                        h1p = h1_live.pop(fp)
                        for d in range(KD):
                            nc.tensor.matmul(
                                py_t[d][:],
                                w2_t[fp][:, d * 128 : (d + 1) * 128],
                                h1p[:],
                                start=(fp == 0),
                                stop=(fp == KF - 1),
                            )
                for d in range(KD):
                    ev = evpool.tile(
                        [128, cn], mybir.dt.float32,
                        tag="ev", name=f"ev{d}_{rep}",
                    )
                    nc.vector.tensor_copy(ev[:], py_t[d][:])
                    nc.sync.dma_start(yT[d, :, c0 : c0 + cn], ev[:])

    nc.compile()
    return nc


def _route_host(hidden_states, w_router):
    """Replicate the reference router on host (f64 -> f32 rounding)."""
    x = np.ascontiguousarray(hidden_states, dtype=np.float32).reshape(T, D)
    l64 = x.astype(np.float64) @ np.asarray(w_router, np.float64)
    logits = l64.astype(np.float32)
    eidx = np.argmax(logits, axis=1).astype(np.int32)
    m = l64.max(axis=1, keepdims=True)
    p = np.exp(l64 - m)
    pmax = (p.max(axis=1) / p.sum(axis=1)).astype(np.float32)
    return x, logits, eidx, pmax


def prepare(hidden_states, w_router, w1, w2, reps=1):
    """Host routing + input sharding + program build (no execution)."""
    x, logits, eidx, pmax = _route_host(hidden_states, w_router)

    order = np.argsort(eidx, kind="stable")
    counts = np.bincount(eidx, minlength=E)
    C = max(256, int(math.ceil(counts.max() / 32)) * 32)

    np_io = np.float32
    if MM_MODE == "bf16":
        import ml_dtypes

        np_io = ml_dtypes.bfloat16

    tok_of_expert = np.split(order, np.cumsum(counts)[:-1])
    in_maps = []
    w1r = np.ascontiguousarray(w1, np.float32).reshape(E, KD, 128, F)
    w2r = np.ascontiguousarray(w2, np.float32).reshape(E, KF, 128, D)
    for e in range(E):
        xe = np.zeros((D, C), np.float32)
        xe[:, : counts[e]] = x[tok_of_expert[e]].T
        in_maps.append(
            {
                "xT": xe.reshape(KD, 128, C).astype(np_io),
                "w1": w1r[e].astype(np_io),
                "w2": w2r[e].astype(np_io),
            }
        )

    nc = _build_program(C, reps=reps)
    meta = dict(
        C=C, counts=counts, tok_of_expert=tok_of_expert, logits=logits,
        eidx=eidx, pmax=pmax,
    )
    return nc, in_maps, meta


def finish(results, meta):
    """Gather per-core outputs back into the full [B,S,D] output."""
    C, counts, tok_of_expert = meta["C"], meta["counts"], meta["tok_of_expert"]
    y = np.empty((T, D), np.float32)
    for e in range(E):
        yT = np.asarray(results[e]["yT"], np.float32)
        y[tok_of_expert[e]] = yT.reshape(D, C).T[: counts[e]]
    out = (meta["pmax"][:, None] * y).reshape(B, S, D).astype(np.float32)
    return out


def run(hidden_states, w_router, w1, w2, trace=False, trace_cores=None):
    nc, in_maps, meta = prepare(hidden_states, w_router, w1, w2)
    res = run_bass_kernel_spmd(
        nc,
        in_maps,
        list(range(NCORES)),
        trace=trace,
        trace_cores=trace_cores,
    )

    out = finish(res.results, meta)
    return (
        (out, meta["logits"].reshape(B, S, E), meta["eidx"].reshape(B, S)),
        res,
    )


def kernel(hidden_states, w_router, w1, w2):
    outs, _ = run(hidden_states, w_router, w1, w2)
    return outs
